# revision 1
# baseline (speedup 1.0000x reference)
"""Trainium2 Bass kernel for ContextQueryAttn (BiDAF-style trilinear attention).

Computes, per batch b:
    sim = sc[:,None] + sq[None,:] + (ctx*wm) @ query.T          (Lc, Lq)
    sim = where(cmask[:,None] | qmask[None,:], -1e30, sim)
    S   = softmax(sim, axis=-1)   (row softmax over Lq)
    SS  = softmax(sim, axis=0)    (col softmax over Lc)
    A   = S @ query               (Lc, D)
    T   = SS.T @ ctx              (Lq, D)
    B   = S @ T                   (Lc, D)
returns (A, B).

Strategy: data-parallel over batch B=32 across 8 cores (4 batches/core).
All matmuls on PE in float32r (fp22 mantissa, full speed at N>=256).
Softmaxes use no max-subtraction (logits are O(+-10); masked entries hit
exp(-1e30)=0 exactly); normalizers come from ones-columns appended to the
matmul RHS operands; fully-masked rows/cols reproduce the reference's
uniform-softmax semantics via ACT scale/bias folding and a predicated copy.
"""

import os
import numpy as np

import concourse.bass as bass
import concourse.tile as tile
from concourse import bacc, mybir
from concourse.bass_utils import run_bass_kernel_spmd

F32 = mybir.dt.float32
F32R = mybir.dt.float32r
EXP = mybir.ActivationFunctionType.Exp
ALU = mybir.AluOpType

B, LC, LQ, D = 32, 2048, 256, 256
NCORES = 8
BPC = B // NCORES          # batches per core
NCT = LC // 128            # 16 context tiles
NQT = LQ // 128            # 2 query tiles
NKD = D // 128             # 2 contraction chunks over D
NCH = LC // 512            # 4 dotT column chunks
NEG = np.float32(-1e30)

# Transposes in plain f32 (2 cyc/row) by default; f32r (1.5) is an option.
TRANSPOSE_DT = F32R


def _build_kernel(tc, nc, ins, outs):
    import contextlib
    ctx = contextlib.ExitStack()

    sb = lambda name, bufs: ctx.enter_context(
        tc.tile_pool(name=name, bufs=bufs))
    ps_pool = ctx.enter_context(tc.tile_pool(name="ps", bufs=6, space="PSUM"))
    t_pool = ctx.enter_context(tc.tile_pool(name="tps", bufs=1, space="PSUM"))

    p_const = sb("const", 1)
    p_ctx = sb("pctx", 2)
    p_ctxT = sb("pctxT", 2)
    p_PT = sb("pPT", 2)
    p_Pc = sb("pPc", 2)
    p_q = sb("pq", 2)
    p_qwmT = sb("pqwmT", 2)
    p_Tn = sb("pTn", 2)
    p_cm = sb("pcm", 2)
    p_cs = sb("pcs", 2)
    p_vec = sb("pvec", 2)
    p_stage = sb("pstage", 8)

    ident = p_const.tile([128, 128], F32R, name="ident")
    nc.sync.dma_start(out=ident[:], in_=ins["ident"])

    r128 = lambda ap: ap.rearrange("(t p) x -> p t x", p=128)
    v128 = lambda ap: ap.rearrange("(t p) -> p t", p=128)

    for b in range(BPC):
        # ---- loads ----
        ctx_sb = p_ctx.tile([128, NCT, 258], F32R, name="ctx_sb")
        nc.sync.dma_start(out=ctx_sb[:], in_=r128(ins["ctx_ext"][b]))
        q_sb = p_q.tile([128, NQT, 258], F32R, name="q_sb")
        nc.sync.dma_start(out=q_sb[:], in_=r128(ins["query_ext"][b]))
        qwmT_sb = p_qwmT.tile([128, NKD, LQ], F32R, name="qwmT_sb")
        nc.sync.dma_start(out=qwmT_sb[:], in_=r128(ins["qwmT"][b]))

        sqb_sb = p_vec.tile([128, NQT], F32, name="sqb_sb")
        nc.sync.dma_start(out=sqb_sb[:], in_=v128(ins["sq_bias"][b]))
        nbs_sb = p_vec.tile([128, NQT], F32, name="nbs_sb")
        nc.sync.dma_start(out=nbs_sb[:], in_=v128(ins["nbs"][b]))
        qsc_sb = p_vec.tile([128, NQT], F32, name="qsc_sb")
        nc.sync.dma_start(out=qsc_sb[:], in_=v128(ins["q_scale"][b]))
        qf_sb = p_vec.tile([128, NQT], F32, name="qf_sb")
        nc.sync.dma_start(out=qf_sb[:], in_=v128(ins["qf"][b]))
        scb_sb = p_vec.tile([128, NCT], F32, name="scb_sb")
        nc.sync.dma_start(out=scb_sb[:], in_=v128(ins["sc_bias"][b]))
        csc_sb = p_vec.tile([128, NCT], F32, name="csc_sb")
        nc.sync.dma_start(out=csc_sb[:], in_=v128(ins["c_scale"][b]))

        cmaskb_sb = p_cm.tile([128, LC], mybir.dt.uint8, name="cmaskb_sb")
        nc.sync.dma_start(out=cmaskb_sb[:],
                          in_=ins["cmask"][b][None, :].to_broadcast((128, LC)))
        ctxsum_sb = p_cs.tile([128, 258], F32, name="ctxsum_sb")
        nc.sync.dma_start(out=ctxsum_sb[:],
                          in_=ins["ctxsum_ext"][b][None, :].to_broadcast((128, 258)))

        rsrec_sb = p_vec.tile([128, NCT], F32, name="rsrec_sb")
        csrec_sb = p_vec.tile([128, NQT], F32, name="csrec_sb")

        # ---- ctx transposes: ctxT[kd] = ctx[:, kd-block].T  (d on partitions)
        ctxT_sb = p_ctxT.tile([128, NKD, LC], F32R, name="ctxT_sb")
        for kd in range(NKD):
            for g in range(NCH):
                tp = ps_pool.tile([128, 512], F32R, tag="ps", name="tp")
                for j in range(4):
                    ci = 4 * g + j
                    nc.tensor.transpose(
                        out=tp[:, bass.ts(j, 128)],
                        in_=ctx_sb[:, ci, bass.ts(kd, 128)],
                        identity=ident[:])
                nc.scalar.copy(ctxT_sb[:, kd, bass.ts(g, 512)], tp[:])

        # ---- row path: dotT (q, c) -> exp -> P^T, cmask predicated copy
        PT_sb = p_PT.tile([128, NQT, LC], F32R, name="PT_sb")
        for qt in range(NQT):
            for ch in range(NCH):
                dt_ps = ps_pool.tile([128, 512], F32, tag="ps", name="dt_ps")
                for kd in range(NKD):
                    nc.tensor.matmul(
                        dt_ps[:],
                        lhsT=qwmT_sb[:, kd, bass.ts(qt, 128)],
                        rhs=ctxT_sb[:, kd, bass.ts(ch, 512)],
                        start=(kd == 0), stop=(kd == NKD - 1))
                # cmasked columns -> -sq_bias[q], cancelling the exp bias
                # exactly: exp(0)=1 (uniform row), incl. qmasked rows where
                # +1e30 + (-1e30) = 0.
                nc.vector.copy_predicated(
                    out=dt_ps[:], mask=cmaskb_sb[:, bass.ts(ch, 512)],
                    data=nbs_sb[:, qt:qt + 1].to_broadcast((128, 512)))
                nc.scalar.activation(
                    PT_sb[:, qt, bass.ts(ch, 512)], dt_ps[:], EXP,
                    bias=sqb_sb[:, qt:qt + 1])

        # ---- col path: dot (c, q) -> exp -> Pc; T accumulation
        T_ps = [t_pool.tile([128, 258], F32, name=f"T_ps{qt}") for qt in range(NQT)]
        Pc_sb = p_Pc.tile([128, NCT, LQ], F32R, name="Pc_sb")
        for ci in range(NCT):
            dps = ps_pool.tile([128, LQ], F32, tag="ps", name="dps")
            for kd in range(NKD):
                nc.tensor.matmul(
                    dps[:],
                    lhsT=ctxT_sb[:, kd, bass.ts(ci, 128)],
                    rhs=qwmT_sb[:, kd, :],
                    start=(kd == 0), stop=(kd == NKD - 1))
            nc.scalar.activation(
                Pc_sb[:, ci, :], dps[:], EXP,
                bias=scb_sb[:, ci:ci + 1], scale=csc_sb[:, ci:ci + 1])
            for qt in range(NQT):
                nc.tensor.matmul(
                    T_ps[qt][:],
                    lhsT=Pc_sb[:, ci, bass.ts(qt, 128)],
                    rhs=ctx_sb[:, ci, :],
                    start=(ci == 0), stop=(ci == NCT - 1))
            # A path interleaved: independent PE work while ACT runs exps
            a_ps = ps_pool.tile([128, 258], F32, tag="ps", name="a_ps")
            for qt in range(NQT):
                nc.tensor.matmul(
                    a_ps[:],
                    lhsT=PT_sb[:, qt, bass.ts(ci, 128)],
                    rhs=q_sb[:, qt, :],
                    start=(qt == 0), stop=(qt == NQT - 1))
            nc.vector.reciprocal(rsrec_sb[:, ci:ci + 1], a_ps[:, 256:257])
            a_st = p_stage.tile([128, 256], F32, tag="ast", name="a_st")
            nc.scalar.mul(a_st[:], a_ps[:, 0:256], rsrec_sb[:, ci:ci + 1])
            nc.sync.dma_start(out=outs["A"][b, bass.ts(ci, 128), :], in_=a_st[:])

        # ---- T finalize: blend qmask + normalize
        Tn_sb = p_Tn.tile([128, NQT, 256], F32R, name="Tn_sb")
        for qt in range(NQT):
            nc.vector.tensor_scalar_mul(
                T_ps[qt][:], T_ps[qt][:], qsc_sb[:, qt:qt + 1])
            nc.vector.scalar_tensor_tensor(
                out=T_ps[qt][:], in0=ctxsum_sb[:], scalar=qf_sb[:, qt:qt + 1],
                in1=T_ps[qt][:], op0=ALU.mult, op1=ALU.add)
            nc.vector.reciprocal(csrec_sb[:, qt:qt + 1], T_ps[qt][:, 256:257])
            nc.scalar.mul(Tn_sb[:, qt, :], T_ps[qt][:, 0:256],
                          csrec_sb[:, qt:qt + 1])

        # ---- Bmat = S @ T
        for ci in range(NCT):
            b_ps = ps_pool.tile([128, 256], F32, tag="ps", name="b_ps")
            for qt in range(NQT):
                nc.tensor.matmul(
                    b_ps[:],
                    lhsT=PT_sb[:, qt, bass.ts(ci, 128)],
                    rhs=Tn_sb[:, qt, :],
                    start=(qt == 0), stop=(qt == NQT - 1))
            b_st = p_stage.tile([128, 256], F32, tag="bst", name="b_st")
            nc.vector.tensor_scalar_mul(b_st[:], b_ps[:], rsrec_sb[:, ci:ci + 1])
            nc.sync.dma_start(out=outs["Bm"][b, bass.ts(ci, 128), :], in_=b_st[:])

    ctx.close()


def build_program():
    nc = bacc.Bacc("TRN2", target_bir_lowering=False, debug=False,
                   num_devices=NCORES)
    ins = {
        "ctx_ext": nc.dram_tensor("ctx_ext", [BPC, LC, 258], F32R,
                                  kind="ExternalInput").ap(),
        "query_ext": nc.dram_tensor("query_ext", [BPC, LQ, 258], F32R,
                                    kind="ExternalInput").ap(),
        "qwmT": nc.dram_tensor("qwmT", [BPC, D, LQ], F32R,
                               kind="ExternalInput").ap(),
        "sq_bias": nc.dram_tensor("sq_bias", [BPC, LQ], F32,
                                  kind="ExternalInput").ap(),
        "q_scale": nc.dram_tensor("q_scale", [BPC, LQ], F32,
                                  kind="ExternalInput").ap(),
        "qf": nc.dram_tensor("qf", [BPC, LQ], F32, kind="ExternalInput").ap(),
        "sc_bias": nc.dram_tensor("sc_bias", [BPC, LC], F32,
                                  kind="ExternalInput").ap(),
        "c_scale": nc.dram_tensor("c_scale", [BPC, LC], F32,
                                  kind="ExternalInput").ap(),
        "cmask": nc.dram_tensor("cmask", [BPC, LC], mybir.dt.uint8,
                                kind="ExternalInput").ap(),
        "ctxsum_ext": nc.dram_tensor("ctxsum_ext", [BPC, 258], F32,
                                     kind="ExternalInput").ap(),
        "nbs": nc.dram_tensor("nbs", [BPC, LQ], F32,
                              kind="ExternalInput").ap(),
        "ident": nc.dram_tensor("ident", [128, 128], F32R,
                                kind="ExternalInput").ap(),
    }
    outs = {
        "A": nc.dram_tensor("A", [BPC, LC, D], F32, kind="ExternalOutput").ap(),
        "Bm": nc.dram_tensor("Bm", [BPC, LC, D], F32, kind="ExternalOutput").ap(),
    }
    with tile.TileContext(nc) as tc:
        _build_kernel(tc, nc, ins, outs)
    nc.compile()
    return nc


def host_prep(context, query, context_mask, query_mask, w0):
    """Host-side preprocessing: shard + build auxiliary tensors (all O(B*L*D))."""
    f = np.float32
    context = np.ascontiguousarray(context, dtype=f)
    query = np.ascontiguousarray(query, dtype=f)
    w0 = np.asarray(w0, dtype=f)
    wc, wq, wm = w0[:D], w0[D:2 * D], w0[2 * D:]
    cf = context_mask.astype(f)
    qf = query_mask.astype(f)
    sc = context @ wc                      # (B, LC)
    sq = query @ wq                        # (B, LQ)
    qwmT = np.ascontiguousarray((query * wm).transpose(0, 2, 1))
    ones_c = np.ones((B, LC, 1), f)
    ones_q = np.ones((B, LQ, 1), f)
    zc = np.zeros((B, LC, 1), f)
    zq = np.zeros((B, LQ, 1), f)
    ctx_ext = np.ascontiguousarray(np.concatenate([context, ones_c, zc], -1))
    query_ext = np.ascontiguousarray(np.concatenate([query, ones_q, zq], -1))
    ctxsum_ext = np.concatenate(
        [context.sum(1, dtype=f), np.full((B, 1), LC, f),
         np.zeros((B, 1), f)], -1)
    q_scale = (1.0 - qf).astype(f)
    sq_bias = (q_scale * sq + qf * NEG).astype(f)
    c_scale = (1.0 - cf).astype(f)
    sc_bias = (c_scale * sc + cf * NEG).astype(f)

    full = {
        "ctx_ext": ctx_ext, "query_ext": query_ext, "qwmT": qwmT,
        "sq_bias": sq_bias, "nbs": -sq_bias, "q_scale": q_scale, "qf": qf,
        "sc_bias": sc_bias, "c_scale": c_scale,
        "cmask": cf.astype(np.uint8),
        "ctxsum_ext": ctxsum_ext,
    }
    const = {"ident": np.eye(128, dtype=f)}
    in_maps = []
    for c in range(NCORES):
        sl = slice(c * BPC, (c + 1) * BPC)
        m = {k: np.ascontiguousarray(v[sl]) for k, v in full.items()}
        m.update(const)
        in_maps.append(m)
    return in_maps


_cached_nc = None


def get_program():
    global _cached_nc
    if _cached_nc is None:
        _cached_nc = build_program()
    return _cached_nc


def run_on_hw(in_maps, **kwargs):
    nc = get_program()
    return run_bass_kernel_spmd(nc, in_maps, core_ids=list(range(NCORES)),
                                **kwargs)


def kernel(context, query, context_mask, query_mask, w0):
    in_maps = host_prep(context, query, context_mask, query_mask, w0)
    res = run_on_hw(in_maps)
    A = np.concatenate([res.results[c]["A"] for c in range(NCORES)], 0)
    Bm = np.concatenate([res.results[c]["Bm"] for c in range(NCORES)], 0)
    return A, Bm



# revision 3
# speedup vs baseline: 1.0309x; 1.0309x over previous
"""Trainium2 Bass kernel for ContextQueryAttn (BiDAF-style trilinear attention).

Computes, per batch b:
    sim = sc[:,None] + sq[None,:] + (ctx*wm) @ query.T          (Lc, Lq)
    sim = where(cmask[:,None] | qmask[None,:], -1e30, sim)
    S   = softmax(sim, axis=-1)   (row softmax over Lq)
    SS  = softmax(sim, axis=0)    (col softmax over Lc)
    A   = S @ query               (Lc, D)
    T   = SS.T @ ctx              (Lq, D)
    B   = S @ T                   (Lc, D)
returns (A, B).

Strategy: data-parallel over batch B=32 across 8 cores (4 batches/core).
v2 changes vs v1 baseline (185.9us):
  - bf16 matmul operands everywhere (f32 PSUM accumulation); bf16 outputs,
    upcast on host. Halves DMA bytes and SBUF footprint.
  - ctxT precomputed on host (kills 128 PE transposes/core, ~10us PE).
  - host-packed SBUF-native layouts; one fat DMA per tensor per batch;
    outputs staged in SBUF and stored with one DMA per tensor per batch
    (sync-engine DMA issue is ~600ns each, serial).
  - cmask override moved post-exp onto SBUF bf16 (copy_predicated data=1.0),
    replacing the pre-exp PSUM nbs trick.
  - software pipelining: B-matmul loop of batch b runs between the row and
    col phases of batch b+1, covering the T-finalize dependency stall.
  - col loop skews T/A matmuls one ci behind the dot matmuls so the PE
    never waits on the ACT exp of the current ci.
Softmaxes use no max-subtraction (logits are O(+-10); masked entries hit
exp(-1e30)=0 exactly); normalizers come from ones-columns appended to the
matmul RHS operands; fully-masked rows/cols reproduce the reference's
uniform-softmax semantics via ACT scale/bias folding, a post-exp predicated
fill of 1.0 (uniform rows), and the q_scale/qf blend of T with ctxsum.
"""

import numpy as np
import ml_dtypes

import concourse.bass as bass
import concourse.tile as tile
from concourse import bacc, mybir
from concourse.bass_utils import run_bass_kernel_spmd

F32 = mybir.dt.float32
BF16 = mybir.dt.bfloat16
U8 = mybir.dt.uint8
EXP = mybir.ActivationFunctionType.Exp
ALU = mybir.AluOpType
NPBF = ml_dtypes.bfloat16

B, LC, LQ, D = 32, 2048, 256, 256
NCORES = 8
BPC = B // NCORES          # batches per core
NCT = LC // 128            # 16 context tiles
NQT = LQ // 128            # 2 query tiles
NKD = D // 128             # 2 contraction chunks over D
NCH = LC // 512            # 4 row-path column chunks
NEG = np.float32(-1e30)
# vecs layout (free dim offsets): sqb[0:2], qsc[2:4], qf[4:6], scb[6:22], csc[22:38]
VW = 38


def _build_kernel(tc, nc, ins, outs):
    import contextlib
    ctx = contextlib.ExitStack()

    sb = lambda name, bufs: ctx.enter_context(
        tc.tile_pool(name=name, bufs=bufs))
    ps_pool = ctx.enter_context(tc.tile_pool(name="ps", bufs=6, space="PSUM"))
    t_pool = ctx.enter_context(tc.tile_pool(name="tps", bufs=1, space="PSUM"))

    p_ctx = sb("pctx", 2)
    p_ctxT = sb("pctxT", 2)
    p_PT = sb("pPT", 2)
    p_Pc = sb("pPc", 2)
    p_q = sb("pq", 2)
    p_qwmT = sb("pqwmT", 2)
    p_Tn = sb("pTn", 2)
    p_cm = sb("pcm", 2)
    p_cs = sb("pcs", 2)
    p_vec = sb("pvec", 2)
    p_ast = sb("past", 2)
    p_bst = sb("pbst", 2)

    # state carried from batch b-1 into the pipelined B loop
    prev = None

    for b in range(BPC):
        # ---- loads (host-packed layouts; one DMA per tensor) ----
        vec_sb = p_vec.tile([128, VW], F32, name="vec_sb")
        nc.sync.dma_start(out=vec_sb[:], in_=ins["vecs"][b])
        qwmT_sb = p_qwmT.tile([128, NKD, LQ], BF16, name="qwmT_sb")
        nc.sync.dma_start(out=qwmT_sb[:], in_=ins["qwmT"][b])
        ctxT_sb = p_ctxT.tile([128, NKD, LC], BF16, name="ctxT_sb")
        for kd in range(NKD):
            nc.sync.dma_start(out=ctxT_sb[:, kd], in_=ins["ctxT"][b][:, kd])
        cm_sb = p_cm.tile([128, LC], mybir.dt.uint16, name="cm_sb")
        nc.sync.dma_start(out=cm_sb[:],
                          in_=ins["cmask"][b][None, :].to_broadcast((128, LC)))
        q_sb = p_q.tile([128, NQT, 258], BF16, name="q_sb")
        nc.sync.dma_start(out=q_sb[:], in_=ins["qext"][b])
        ctx_sb = p_ctx.tile([128, NCT, 258], BF16, name="ctx_sb")
        nc.sync.dma_start(out=ctx_sb[:], in_=ins["ctx"][b])
        cs_sb = p_cs.tile([128, 258], F32, name="cs_sb")
        nc.sync.dma_start(out=cs_sb[:],
                          in_=ins["ctxsum"][b][None, :].to_broadcast((128, 258)))

        sqb = lambda qt: vec_sb[:, 0 + qt:1 + qt]
        qsc = lambda qt: vec_sb[:, 2 + qt:3 + qt]
        qfv = lambda qt: vec_sb[:, 4 + qt:5 + qt]
        scb = lambda ci: vec_sb[:, 6 + ci:7 + ci]
        csc = lambda ci: vec_sb[:, 22 + ci:23 + ci]

        one_sb = p_vec.tile([128, 1], BF16, name="one_sb")
        nc.vector.memset(one_sb[:], 1.0)

        rsrec_sb = p_vec.tile([128, NCT], F32, name="rsrec_sb")
        csrec_sb = p_vec.tile([128, NQT], F32, name="csrec_sb")

        # ---- row path: dotT (q, c) -> exp -> P^T; post-exp cmask fill of 1.0
        PT_sb = p_PT.tile([128, NQT, LC], BF16, name="PT_sb")
        for ch in range(NCH):
            for qt in range(NQT):
                dt_ps = ps_pool.tile([128, 512], F32, tag="ps", name="dt_ps")
                for kd in range(NKD):
                    nc.tensor.matmul(
                        dt_ps[:],
                        lhsT=qwmT_sb[:, kd, bass.ts(qt, 128)],
                        rhs=ctxT_sb[:, kd, bass.ts(ch, 512)],
                        start=(kd == 0), stop=(kd == NKD - 1))
                nc.scalar.activation(
                    PT_sb[:, qt, bass.ts(ch, 512)], dt_ps[:], EXP,
                    bias=sqb(qt))
                # cmasked columns -> 1.0 (uniform row; includes qmasked rows,
                # matching jax softmax's max-subtraction semantics)
                nc.vector.copy_predicated(
                    out=PT_sb[:, qt, bass.ts(ch, 512)],
                    mask=cm_sb[:, bass.ts(ch, 512)],
                    data=one_sb[:].to_broadcast((128, 512)))

        # ---- pipelined B loop of previous batch (covers T-finalize stall)
        if prev is not None:
            pPT, pTn, prsrec, pb = prev
            for ci in range(NCT):
                b_ps = ps_pool.tile([128, 256], F32, tag="ps", name="b_ps")
                for qt in range(NQT):
                    nc.tensor.matmul(
                        b_ps[:],
                        lhsT=pPT[:, qt, bass.ts(ci, 128)],
                        rhs=pTn[:, qt, :],
                        start=(qt == 0), stop=(qt == NQT - 1))
                b_st = p_bst.tile([128, 256], BF16, tag="bst", name="b_st")
                nc.vector.tensor_scalar_mul(b_st[:], b_ps[:],
                                            prsrec[:, ci:ci + 1])
                nc.sync.dma_start(
                    out=outs["Bm"][pb, bass.ts(ci, 128), :], in_=b_st[:])

        # ---- col path: dot (c, q) -> exp -> Pc; T accumulation; A path
        # T/A matmuls run one ci behind the dot so PE never waits on exp.
        T_ps = [t_pool.tile([128, 258], F32, name=f"T_ps{qt}")
                for qt in range(NQT)]
        A_st = p_ast.tile([128, NCT, 256], BF16, name="A_st")
        Pc_sb = p_Pc.tile([128, NCT, LQ], BF16, name="Pc_sb")

        def dot_ci(ci):
            dps = ps_pool.tile([128, LQ], F32, tag="ps", name="dps")
            for kd in range(NKD):
                nc.tensor.matmul(
                    dps[:],
                    lhsT=ctxT_sb[:, kd, bass.ts(ci, 128)],
                    rhs=qwmT_sb[:, kd, :],
                    start=(kd == 0), stop=(kd == NKD - 1))
            nc.scalar.activation(
                Pc_sb[:, ci, :], dps[:], EXP,
                bias=scb(ci), scale=csc(ci))

        def ta_ci(ci):
            for qt in range(NQT):
                nc.tensor.matmul(
                    T_ps[qt][:],
                    lhsT=Pc_sb[:, ci, bass.ts(qt, 128)],
                    rhs=ctx_sb[:, ci, :],
                    start=(ci == 0), stop=(ci == NCT - 1))
            a_ps = ps_pool.tile([128, 258], F32, tag="ps", name="a_ps")
            for qt in range(NQT):
                nc.tensor.matmul(
                    a_ps[:],
                    lhsT=PT_sb[:, qt, bass.ts(ci, 128)],
                    rhs=q_sb[:, qt, :],
                    start=(qt == 0), stop=(qt == NQT - 1))
            nc.vector.reciprocal(rsrec_sb[:, ci:ci + 1], a_ps[:, 256:257])
            nc.scalar.mul(A_st[:, ci, :], a_ps[:, 0:256],
                          rsrec_sb[:, ci:ci + 1])

        dot_ci(0)
        for ci in range(1, NCT):
            dot_ci(ci)
            ta_ci(ci - 1)
        ta_ci(NCT - 1)
        nc.sync.dma_start(
            out=outs["A"][b].rearrange("(t p) x -> p t x", p=128),
            in_=A_st[:])

        # ---- T finalize: blend qmask + normalize
        Tn_sb = p_Tn.tile([128, NQT, 256], BF16, name="Tn_sb")
        for qt in range(NQT):
            nc.vector.tensor_scalar_mul(
                T_ps[qt][:], T_ps[qt][:], qsc(qt))
            nc.vector.scalar_tensor_tensor(
                out=T_ps[qt][:], in0=cs_sb[:], scalar=qfv(qt),
                in1=T_ps[qt][:], op0=ALU.mult, op1=ALU.add)
            nc.vector.reciprocal(csrec_sb[:, qt:qt + 1], T_ps[qt][:, 256:257])
            nc.scalar.mul(Tn_sb[:, qt, :], T_ps[qt][:, 0:256],
                          csrec_sb[:, qt:qt + 1])

        prev = (PT_sb, Tn_sb, rsrec_sb, b)

    # ---- drain: B loop of the last batch
    pPT, pTn, prsrec, pb = prev
    for ci in range(NCT):
        b_ps = ps_pool.tile([128, 256], F32, tag="ps", name="b_ps")
        for qt in range(NQT):
            nc.tensor.matmul(
                b_ps[:],
                lhsT=pPT[:, qt, bass.ts(ci, 128)],
                rhs=pTn[:, qt, :],
                start=(qt == 0), stop=(qt == NQT - 1))
        b_st = p_bst.tile([128, 256], BF16, tag="bst", name="b_st")
        nc.vector.tensor_scalar_mul(b_st[:], b_ps[:], prsrec[:, ci:ci + 1])
        nc.sync.dma_start(out=outs["Bm"][pb, bass.ts(ci, 128), :], in_=b_st[:])

    ctx.close()


def build_program():
    nc = bacc.Bacc("TRN2", target_bir_lowering=False, debug=False,
                   num_devices=NCORES)
    ins = {
        "ctx": nc.dram_tensor("ctx", [BPC, 128, NCT, 258], BF16,
                              kind="ExternalInput").ap(),
        "ctxT": nc.dram_tensor("ctxT", [BPC, 128, NKD, LC], BF16,
                               kind="ExternalInput").ap(),
        "qext": nc.dram_tensor("qext", [BPC, 128, NQT, 258], BF16,
                               kind="ExternalInput").ap(),
        "qwmT": nc.dram_tensor("qwmT", [BPC, 128, NKD, LQ], BF16,
                               kind="ExternalInput").ap(),
        "vecs": nc.dram_tensor("vecs", [BPC, 128, VW], F32,
                               kind="ExternalInput").ap(),
        "cmask": nc.dram_tensor("cmask", [BPC, LC], mybir.dt.uint16,
                                kind="ExternalInput").ap(),
        "ctxsum": nc.dram_tensor("ctxsum", [BPC, 258], F32,
                                 kind="ExternalInput").ap(),
    }
    outs = {
        "A": nc.dram_tensor("A", [BPC, LC, D], BF16,
                            kind="ExternalOutput").ap(),
        "Bm": nc.dram_tensor("Bm", [BPC, LC, D], BF16,
                             kind="ExternalOutput").ap(),
    }
    with tile.TileContext(nc) as tc:
        _build_kernel(tc, nc, ins, outs)
    nc.compile()
    return nc


def host_prep(context, query, context_mask, query_mask, w0):
    """Host-side preprocessing: shard + pack SBUF-native layouts."""
    f = np.float32
    context = np.ascontiguousarray(context, dtype=f)
    query = np.ascontiguousarray(query, dtype=f)
    w0 = np.asarray(w0, dtype=f)
    wc, wq, wm = w0[:D], w0[D:2 * D], w0[2 * D:]
    cf = context_mask.astype(f)
    qf = query_mask.astype(f)
    sc = context @ wc                      # (B, LC)
    sq = query @ wq                        # (B, LQ)

    ones_c = np.ones((B, LC, 1), f)
    ones_q = np.ones((B, LQ, 1), f)
    zc = np.zeros((B, LC, 1), f)
    zq = np.zeros((B, LQ, 1), f)
    # ctx packed [B, 128, NCT, 258]: val[p,t,x] = ctx_ext[t*128+p, x]
    ctx_ext = np.concatenate([context, ones_c, zc], -1)
    ctx_p = np.ascontiguousarray(
        ctx_ext.reshape(B, NCT, 128, 258).transpose(0, 2, 1, 3)).astype(NPBF)
    # qext packed [B, 128, NQT, 258]
    q_ext = np.concatenate([query, ones_q, zq], -1)
    qext_p = np.ascontiguousarray(
        q_ext.reshape(B, NQT, 128, 258).transpose(0, 2, 1, 3)).astype(NPBF)
    # ctxT packed [B, 128, NKD, LC]: val[p,kd,c] = ctx[c, kd*128+p]
    ctxT_p = np.ascontiguousarray(
        context.reshape(B, LC, NKD, 128).transpose(0, 3, 2, 1)).astype(NPBF)
    # qwmT packed [B, 128, NKD, LQ]: val[p,kd,q] = (query*wm)[q, kd*128+p]
    qwm = query * wm
    qwmT_p = np.ascontiguousarray(
        qwm.reshape(B, LQ, NKD, 128).transpose(0, 3, 2, 1)).astype(NPBF)

    # vecs [B, 128, 38]: sqb(2) qsc(2) qf(2) scb(16) csc(16); val[p,t]=v[t*128+p]
    q_scale = (1.0 - qf).astype(f)
    sq_bias = (q_scale * sq + qf * NEG).astype(f)
    c_scale = (1.0 - cf).astype(f)
    sc_bias = (c_scale * sc + cf * NEG).astype(f)
    pq = lambda v: v.reshape(B, NQT, 128).transpose(0, 2, 1)
    pc = lambda v: v.reshape(B, NCT, 128).transpose(0, 2, 1)
    vecs = np.ascontiguousarray(np.concatenate(
        [pq(sq_bias), pq(q_scale), pq(qf), pc(sc_bias), pc(c_scale)], -1))

    ctxsum = np.concatenate(
        [context.sum(1, dtype=f), np.full((B, 1), LC, f),
         np.zeros((B, 1), f)], -1)

    full = {
        "ctx": ctx_p, "ctxT": ctxT_p, "qext": qext_p, "qwmT": qwmT_p,
        "vecs": vecs, "cmask": cf.astype(np.uint16), "ctxsum": ctxsum,
    }
    in_maps = []
    for c in range(NCORES):
        sl = slice(c * BPC, (c + 1) * BPC)
        m = {k: np.ascontiguousarray(v[sl]) for k, v in full.items()}
        in_maps.append(m)
    return in_maps


_cached_nc = None


def get_program():
    global _cached_nc
    if _cached_nc is None:
        _cached_nc = build_program()
    return _cached_nc


def run_on_hw(in_maps, **kwargs):
    nc = get_program()
    return run_bass_kernel_spmd(nc, in_maps, core_ids=list(range(NCORES)),
                                **kwargs)


def kernel(context, query, context_mask, query_mask, w0):
    in_maps = host_prep(context, query, context_mask, query_mask, w0)
    res = run_on_hw(in_maps)
    A = np.concatenate(
        [np.asarray(res.results[c]["A"]).astype(np.float32)
         for c in range(NCORES)], 0)
    Bm = np.concatenate(
        [np.asarray(res.results[c]["Bm"]).astype(np.float32)
         for c in range(NCORES)], 0)
    return A, Bm


# revision 6
# speedup vs baseline: 1.8750x; 1.8188x over previous
"""Trainium2 Bass kernel for ContextQueryAttn (BiDAF-style trilinear attention).

Computes, per batch b:
    sim = sc[:,None] + sq[None,:] + (ctx*wm) @ query.T          (Lc, Lq)
    sim = where(cmask[:,None] | qmask[None,:], -1e30, sim)
    S   = softmax(sim, axis=-1)   (row softmax over Lq)
    SS  = softmax(sim, axis=0)    (col softmax over Lc)
    A   = S @ query               (Lc, D)
    T   = SS.T @ ctx              (Lq, D)
    B   = S @ T                   (Lc, D)
returns (A, B).

Strategy: data-parallel over batch B=32 across 8 cores (4 batches/core).
bf16 matmul operands, f32 PSUM accumulation, bf16 outputs upcast on host.
The ACT (scalar) engine has ~650ns/instruction nearly independent of width,
so the design minimizes ACT/DVE instruction count:
  - chip ships UNNORMALIZED A_raw = E_row^T? no: A_raw = E_row @ query and
    B_raw = E_row @ Tn plus the row sums; host divides by rowsum.
  - cmasked rows of A/B (uniform softmax rows in the reference) are fixed
    up on the host from query.mean and Tn.mean; no on-chip mask override.
  - col-path numerators need no per-row bias: T = colnorm(E_col) is
    invariant to any per-q column scaling, so Pc = exp(dot) with the
    e^{sc[c]} (and cmask zeroing) folded into a host-scaled ctx' used as
    the T-matmul rhs; qmasked columns are repaired by the qf blend.
  - col dot / A / B matmuls are paired: two 256-wide ci outputs share one
    512-wide PSUM bank, halving exp/drain instruction counts.
  - rowsum via 8 ones-vector matmuls into a [4,512] PSUM tile, 1 drain.
  - software pipelining: B loop of batch b runs inside batch b+1's row
    phase; T/A matmuls skew one ci-pair behind the dot matmuls.
Masked-softmax exactness: no max subtraction (logits O(+-10)); qmask folds
as -1e30 into the row-exp bias so exp=0 exactly; cmask rows excluded from
the col softmax by ctx' = 0; fully-masked T rows replaced via q_scale/qf
blend with ctxsum.
"""

import numpy as np
import ml_dtypes

import concourse.bass as bass
import concourse.tile as tile
from concourse import bacc, mybir
from concourse.bass_utils import run_bass_kernel_spmd

F32 = mybir.dt.float32
BF16 = mybir.dt.bfloat16
EXP = mybir.ActivationFunctionType.Exp
ALU = mybir.AluOpType
NPBF = ml_dtypes.bfloat16

B, LC, LQ, D = 32, 2048, 256, 256
NCORES = 8
BPC = B // NCORES          # batches per core
NCT = LC // 128            # 16 context tiles
NQT = LQ // 128            # 2 query tiles
NKD = D // 128             # 2 contraction chunks over D
NCH = LC // 512            # 4 row-path column chunks
NP = NCT // 2              # 8 ci pairs
NEG = np.float32(-1e30)
# vecs layout (free dim offsets): sqb[0:2], qsc[2:4], qf[4:6]
VW = 6


def _build_kernel(tc, nc, ins, outs):
    import contextlib
    ctx = contextlib.ExitStack()

    sb = lambda name, bufs: ctx.enter_context(
        tc.tile_pool(name=name, bufs=bufs))
    ps_pool = ctx.enter_context(tc.tile_pool(name="ps", bufs=6, space="PSUM"))
    t_pool = ctx.enter_context(tc.tile_pool(name="tps", bufs=1, space="PSUM"))

    p_ctx = sb("pctx", 2)
    p_ctxT = sb("pctxT", 2)
    p_PT = sb("pPT", 2)
    p_Pc = sb("pPc", 2)
    p_q = sb("pq", 2)
    p_qwmT = sb("pqwmT", 2)
    p_Tn = sb("pTn", 2)
    p_cs = sb("pcs", 2)
    p_vec = sb("pvec", 2)
    p_ast = sb("past", 2)
    p_bst = sb("pbst", 2)

    # ---- pipelined B loop of previous batch ----
    def b_loop(pPT, pTn, pb):
        B_st = p_bst.tile([128, NCT, 256], BF16, name="B_st")
        for p in range(NP):
            b_ps = ps_pool.tile([128, 2, 256], F32, tag="ps", name="b_ps")
            for h in range(2):
                for qt in range(NQT):
                    nc.tensor.matmul(
                        b_ps[:, h, :],
                        lhsT=pPT[:, qt, bass.ts(2 * p + h, 128)],
                        rhs=pTn[:, qt, :],
                        start=(qt == 0), stop=(qt == NQT - 1))
            if p % 2 == 0:
                nc.scalar.copy(B_st[:, 2 * p:2 * p + 2, :], b_ps[:])
            else:
                nc.vector.tensor_scalar_add(B_st[:, 2 * p:2 * p + 2, :],
                                            b_ps[:], 0.0)
        nc.sync.dma_start(
            out=outs["Bm"][pb].rearrange("(t p) x -> p t x", p=128),
            in_=B_st[:])

    prev = None

    for b in range(BPC):
        # ---- loads (host-packed layouts; one DMA per tensor) ----
        vec_sb = p_vec.tile([128, VW], F32, name="vec_sb")
        nc.sync.dma_start(out=vec_sb[:], in_=ins["vecs"][b])
        qwmT_sb = p_qwmT.tile([128, NKD, LQ], BF16, name="qwmT_sb")
        nc.sync.dma_start(out=qwmT_sb[:], in_=ins["qwmT"][b])
        ctxT_sb = p_ctxT.tile([128, NKD, LC], BF16, name="ctxT_sb")
        for kd in range(NKD):
            nc.sync.dma_start(out=ctxT_sb[:, kd], in_=ins["ctxT"][b][:, kd])
        q_sb = p_q.tile([128, NQT, 258], BF16, name="q_sb")
        nc.sync.dma_start(out=q_sb[:], in_=ins["qext"][b])
        ctx_sb = p_ctx.tile([128, NCT, 258], BF16, name="ctx_sb")
        nc.sync.dma_start(out=ctx_sb[:], in_=ins["ctx"][b])
        cs_sb = p_cs.tile([128, 258], F32, name="cs_sb")
        nc.sync.dma_start(out=cs_sb[:],
                          in_=ins["ctxsum"][b][None, :].to_broadcast((128, 258)))

        sqb = lambda qt: vec_sb[:, 0 + qt:1 + qt]
        qsc = lambda qt: vec_sb[:, 2 + qt:3 + qt]
        qfv = lambda qt: vec_sb[:, 4 + qt:5 + qt]

        csrec_sb = p_vec.tile([128, NQT], F32, name="csrec_sb")

        # ---- row path: dotT (q, c) -> exp(+sq_bias) -> P^T (= E_row^T)
        # rowsum matmuls skew one chunk behind the exps.
        PT_sb = p_PT.tile([128, NQT, LC], BF16, name="PT_sb")

        def row_ch(ch):
            for qt in range(NQT):
                dt_ps = ps_pool.tile([128, 512], F32, tag="ps", name="dt_ps")
                for kd in range(NKD):
                    nc.tensor.matmul(
                        dt_ps[:],
                        lhsT=qwmT_sb[:, kd, bass.ts(qt, 128)],
                        rhs=ctxT_sb[:, kd, bass.ts(ch, 512)],
                        start=(kd == 0), stop=(kd == NKD - 1))
                nc.scalar.activation(
                    PT_sb[:, qt, bass.ts(ch, 512)], dt_ps[:], EXP,
                    bias=sqb(qt))

        for ch in range(NCH):
            row_ch(ch)

        # ---- B loop of previous batch fills the PE while ACT runs exps
        if prev is not None:
            b_loop(*prev)

        # ---- col path: paired dot -> exp (no bias) -> Pc; T accum; A pairs
        T_ps = [t_pool.tile([128, 258], F32, name=f"T_ps{qt}")
                for qt in range(NQT)]
        A_st = p_ast.tile([128, NCT, 258], BF16, name="A_st")
        Pc_sb = p_Pc.tile([128, NCT, LQ], BF16, name="Pc_sb")

        def dot_p(p):
            dps = ps_pool.tile([128, 2, 256], F32, tag="ps", name="dps")
            for h in range(2):
                for kd in range(NKD):
                    nc.tensor.matmul(
                        dps[:, h, :],
                        lhsT=ctxT_sb[:, kd, bass.ts(2 * p + h, 128)],
                        rhs=qwmT_sb[:, kd, :],
                        start=(kd == 0), stop=(kd == NKD - 1))
            nc.scalar.activation(
                Pc_sb[:, 2 * p:2 * p + 2, :], dps[:], EXP)

        def ta_p(p):
            for h in range(2):
                ci = 2 * p + h
                for qt in range(NQT):
                    nc.tensor.matmul(
                        T_ps[qt][:],
                        lhsT=Pc_sb[:, ci, bass.ts(qt, 128)],
                        rhs=ctx_sb[:, ci, :],
                        start=(ci == 0), stop=(ci == NCT - 1))
            for h in range(2):
                ci = 2 * p + h
                a_ps = ps_pool.tile([128, 258], F32, tag="ps", name="a_ps")
                for qt in range(NQT):
                    nc.tensor.matmul(
                        a_ps[:],
                        lhsT=PT_sb[:, qt, bass.ts(ci, 128)],
                        rhs=q_sb[:, qt, :],
                        start=(qt == 0), stop=(qt == NQT - 1))
                if ci % 2 == 0:
                    nc.vector.tensor_scalar_add(A_st[:, ci, :], a_ps[:], 0.0)
                else:
                    nc.scalar.copy(A_st[:, ci, :], a_ps[:])

        dot_p(0)
        for p in range(1, NP):
            dot_p(p)
            ta_p(p - 1)
        ta_p(NP - 1)
        nc.sync.dma_start(
            out=outs["A"][b].rearrange("(t p) x -> p t x", p=128),
            in_=A_st[:])

        # ---- T finalize: blend qmask + normalize -> Tn (bf16), ship Tn
        Tn_sb = p_Tn.tile([128, NQT, 256], BF16, name="Tn_sb")
        for qt in range(NQT):
            nc.vector.tensor_scalar_mul(T_ps[qt][:], T_ps[qt][:], qsc(qt))
            nc.vector.scalar_tensor_tensor(
                out=T_ps[qt][:], in0=cs_sb[:], scalar=qfv(qt),
                in1=T_ps[qt][:], op0=ALU.mult, op1=ALU.add)
            nc.vector.reciprocal(csrec_sb[:, qt:qt + 1], T_ps[qt][:, 256:257])
            nc.scalar.mul(Tn_sb[:, qt, :], T_ps[qt][:, 0:256],
                          csrec_sb[:, qt:qt + 1])
        nc.sync.dma_start(out=outs["Tn"][b], in_=Tn_sb[:])

        prev = (PT_sb, Tn_sb, b)

    b_loop(*prev)
    ctx.close()


def build_program():
    nc = bacc.Bacc("TRN2", target_bir_lowering=False, debug=False,
                   num_devices=NCORES)
    ins = {
        "ctx": nc.dram_tensor("ctx", [BPC, 128, NCT, 258], BF16,
                              kind="ExternalInput").ap(),
        "ctxT": nc.dram_tensor("ctxT", [BPC, 128, NKD, LC], BF16,
                               kind="ExternalInput").ap(),
        "qext": nc.dram_tensor("qext", [BPC, 128, NQT, 258], BF16,
                               kind="ExternalInput").ap(),
        "qwmT": nc.dram_tensor("qwmT", [BPC, 128, NKD, LQ], BF16,
                               kind="ExternalInput").ap(),
        "vecs": nc.dram_tensor("vecs", [BPC, 128, VW], F32,
                               kind="ExternalInput").ap(),
        "ctxsum": nc.dram_tensor("ctxsum", [BPC, 258], F32,
                                 kind="ExternalInput").ap(),
    }
    outs = {
        "A": nc.dram_tensor("A", [BPC, LC, 258], BF16,
                            kind="ExternalOutput").ap(),
        "Bm": nc.dram_tensor("Bm", [BPC, LC, D], BF16,
                             kind="ExternalOutput").ap(),
        "Tn": nc.dram_tensor("Tn", [BPC, 128, NQT, 256], BF16,
                             kind="ExternalOutput").ap(),
    }
    with tile.TileContext(nc) as tc:
        _build_kernel(tc, nc, ins, outs)
    nc.compile()
    return nc


def host_prep(context, query, context_mask, query_mask, w0):
    """Host-side preprocessing: shard + pack SBUF-native layouts."""
    f = np.float32
    context = np.ascontiguousarray(context, dtype=f)
    query = np.ascontiguousarray(query, dtype=f)
    w0 = np.asarray(w0, dtype=f)
    wc, wq, wm = w0[:D], w0[D:2 * D], w0[2 * D:]
    cf = context_mask.astype(f)
    qf = query_mask.astype(f)
    sc = context @ wc                      # (B, LC)
    sq = query @ wq                        # (B, LQ)

    # ctx' = e^{sc[c]} * (1-cf[c]) * [ctx | 1 | 0]  (col-softmax numerator
    # weights; cmasked rows vanish). Packed [B, 128, NCT, 258].
    esc = (np.exp(sc) * (1.0 - cf))[:, :, None].astype(f)
    ones_c = np.ones((B, LC, 1), f)
    zc = np.zeros((B, LC, 1), f)
    ctx_ext = np.concatenate([context, ones_c, zc], -1) * esc
    ctx_p = np.ascontiguousarray(
        ctx_ext.reshape(B, NCT, 128, 258).transpose(0, 2, 1, 3)).astype(NPBF)
    # qext packed [B, 128, NQT, 258]: [query | 1 | 0] (ones col -> rowsum)
    ones_q = np.ones((B, LQ, 1), f)
    zq = np.zeros((B, LQ, 1), f)
    q_ext = np.concatenate([query, ones_q, zq], -1)
    qext_p = np.ascontiguousarray(
        q_ext.reshape(B, NQT, 128, 258).transpose(0, 2, 1, 3)).astype(NPBF)
    # ctxT packed [B, 128, NKD, LC]: val[p,kd,c] = ctx[c, kd*128+p]
    ctxT_p = np.ascontiguousarray(
        context.reshape(B, LC, NKD, 128).transpose(0, 3, 2, 1)).astype(NPBF)
    # qwmT packed [B, 128, NKD, LQ]: val[p,kd,q] = (query*wm)[q, kd*128+p]
    qwm = query * wm
    qwmT_p = np.ascontiguousarray(
        qwm.reshape(B, LQ, NKD, 128).transpose(0, 3, 2, 1)).astype(NPBF)

    # vecs [B, 128, 6]: sqb(2) qsc(2) qf(2); val[p,t] = v[t*128+p]
    q_scale = (1.0 - qf).astype(f)
    sq_bias = (q_scale * sq + qf * NEG).astype(f)
    pq = lambda v: v.reshape(B, NQT, 128).transpose(0, 2, 1)
    vecs = np.ascontiguousarray(np.concatenate(
        [pq(sq_bias), pq(q_scale), pq(qf)], -1))

    ctxsum = np.concatenate(
        [context.sum(1, dtype=f), np.full((B, 1), LC, f),
         np.zeros((B, 1), f)], -1)

    full = {
        "ctx": ctx_p, "ctxT": ctxT_p, "qext": qext_p, "qwmT": qwmT_p,
        "vecs": vecs, "ctxsum": ctxsum,
    }
    in_maps = []
    for c in range(NCORES):
        sl = slice(c * BPC, (c + 1) * BPC)
        m = {k: np.ascontiguousarray(v[sl]) for k, v in full.items()}
        in_maps.append(m)
    return in_maps


_cached_nc = None


def get_program():
    global _cached_nc
    if _cached_nc is None:
        _cached_nc = build_program()
    return _cached_nc


def run_on_hw(in_maps, **kwargs):
    nc = get_program()
    return run_bass_kernel_spmd(nc, in_maps, core_ids=list(range(NCORES)),
                                **kwargs)


def host_post(res, context_mask, query):
    """Normalize by rowsum; overwrite cmasked rows (uniform softmax rows)."""
    f = np.float32
    A_ext = np.concatenate(
        [np.asarray(res.results[c]["A"]).astype(f) for c in range(NCORES)], 0)
    A_raw = A_ext[:, :, 0:256]
    rs = A_ext[:, :, 256]
    B_raw = np.concatenate(
        [np.asarray(res.results[c]["Bm"]).astype(f) for c in range(NCORES)], 0)
    TnD = np.concatenate(
        [np.asarray(res.results[c]["Tn"]).astype(f) for c in range(NCORES)], 0)
    # Tn_full[b, t*128+p, :] = TnD[b, p, t, :]
    Tn_full = TnD.transpose(0, 2, 1, 3).reshape(B, LQ, D)

    cm = np.asarray(context_mask, bool)[:, :, None]
    rs_safe = np.where(cm[:, :, 0], f(1.0), rs)[:, :, None]
    qmean = np.asarray(query, f).mean(1)[:, None, :]
    tmean = Tn_full.mean(1)[:, None, :]
    A = np.where(cm, qmean, A_raw / rs_safe)
    Bm = np.where(cm, tmean, B_raw / rs_safe)
    return A, Bm


def kernel(context, query, context_mask, query_mask, w0):
    in_maps = host_prep(context, query, context_mask, query_mask, w0)
    res = run_on_hw(in_maps)
    return host_post(res, context_mask, query)
